# revision 1
# baseline (speedup 1.0000x reference)
"""Trainium2 Bass kernel for nn_MemoryN2N (vq_codebook).

Self-contained: hardcodes shapes/sharding. Data-parallel over the
n = b*h*w token axis: core m processes batch element m (4096 tokens).
Codebook + MLP weights replicated; segment-sum counts/sums all-reduced.
"""

import numpy as np

# -- problem constants (hardcoded from the problem spec) --
B, C, H, W, K = 8, 256, 64, 64, 2048
CY = 4                 # y channels
CD = C + CY            # 260
CDA = CD + 1           # 261 (+ ones column for counts / sumexp)
HWN = H * W            # 4096 tokens per core
P = 128
KC = K // P            # 16 codebook chunks
NCC = C // P           # 2 channel chunks
NT = HWN // P          # 32 token tiles (pass 1)
GRP = 8                # token tiles per pass-1 group
NGW = 512              # pass-2 token group width
NG2 = HWN // NGW       # 8 pass-2 groups
N_CORES = 8
RATE = 0.999
EPS_CNT = 1e-6

_CACHE = {}


def _build_nc(single_core=False):
    import concourse.bacc as bacc
    import concourse.mybir as mybir
    import concourse.tile as tile

    f32 = mybir.dt.float32
    f32r = mybir.dt.float32r
    bf16 = mybir.dt.bfloat16
    i32 = mybir.dt.int32
    AF = mybir.ActivationFunctionType
    OP = mybir.AluOpType
    AX = mybir.AxisListType

    nc = bacc.Bacc("TRN2", target_bir_lowering=False, debug=False,
                   num_devices=1 if single_core else N_CORES)

    xm = nc.dram_tensor("xm", [C, HWN], f32, kind="ExternalInput").ap()
    ym = nc.dram_tensor("ym", [CY, HWN], f32, kind="ExternalInput").ap()
    fw_d = nc.dram_tensor("feat_w", [K, CD], f32, kind="ExternalInput").ap()
    w1_d = nc.dram_tensor("w1", [CD, C], f32, kind="ExternalInput").ap()
    b1_d = nc.dram_tensor("b1", [C], f32, kind="ExternalInput").ap()
    w2_d = nc.dram_tensor("w2", [C, C], f32, kind="ExternalInput").ap()
    b2_d = nc.dram_tensor("b2", [C], f32, kind="ExternalInput").ap()
    om = nc.dram_tensor("om", [C, HWN], f32, kind="ExternalOutput").ap()

    def r(ap):  # relaxed-fp32 view for PE matmuls
        if ap.dtype == f32r:
            return ap
        return ap.bitcast(f32r)

    from contextlib import ExitStack

    with tile.TileContext(nc) as tc:
        with tc.tile_pool(name="persist", bufs=1) as pp, \
             tc.tile_pool(name="dram", bufs=1, space="DRAM") as dp:
            # ---- persistent tiles ----
            xn = [pp.tile([P, HWN], f32r, name=f"xn{i}") for i in range(NCC)]
            mnT = [pp.tile([P, K], f32r, name=f"mnT{i}") for i in range(NCC)]
            nw = [pp.tile([P, CDA], bf16, name=f"nw{i}") for i in range(KC)]
            sums = [pp.tile([P, CDA], f32, name=f"sums{i}")
                    for i in range(KC)]
            # xyT tiles released after stage 1
            mid = ExitStack()
            mp = mid.enter_context(tc.tile_pool(name="midp", bufs=1))
            xyT = [mp.tile([P, CDA], bf16, name=f"xyT{i}") for i in range(NT)]
            w1s = [pp.tile([P, C], f32r, name="w1s0"),
                   pp.tile([P, C], f32r, name="w1s1"),
                   pp.tile([CY + 1, C], f32r, name="w1s2")]
            w2s = [pp.tile([P, C], f32r, name=f"w2s{i}") for i in range(2)]
            b1s = [pp.tile([P, 1], f32, name=f"b1s{i}") for i in range(2)]
            b2s = [pp.tile([P, 1], f32, name=f"b2s{i}") for i in range(2)]
            ones_col = pp.tile([P, 1], f32r, name="ones_col")
            ones_row = pp.tile([1, P], f32r, name="ones_row")
            ident = pp.tile([P, P], f32, name="ident")

            cc_in = dp.tile([K, CDA], f32, name="cc_in")
            cc_out = dp.tile([K, CDA], f32, name="cc_out",
                             addr_space="Shared")

            # ---- stage 0: constants, weights, codebook prep ----
            ones_f32 = pp.tile([P, 1], f32, name="ones_f32")
            orow_f32 = pp.tile([1, P], f32, name="orow_f32")
            nc.vector.memset(ones_f32[:], 1.0)
            nc.vector.memset(orow_f32[:], 1.0)
            nc.scalar.activation(ones_col[:], ones_f32[:], AF.Copy)
            nc.scalar.activation(ones_row[:], orow_f32[:], AF.Copy)
            iid = pp.tile([P, P], i32, name="iid")
            nc.gpsimd.iota(iid[:], pattern=[[1, P]], base=0,
                           channel_multiplier=-1)
            nc.gpsimd.tensor_scalar(ident[:], iid[:], 0, None, OP.is_equal)

            wstg = [pp.tile([P, C], f32, name=f"wstg{i}") for i in range(5)]
            nc.sync.dma_start(wstg[0][:], w1_d[0:P, :])
            nc.sync.dma_start(wstg[1][:], w1_d[P:2 * P, :])
            nc.vector.memset(wstg[2][0:1, :], 0.0)
            nc.sync.dma_start(wstg[2][1:CY + 1, :], w1_d[2 * P:CD, :])
            nc.sync.dma_start(wstg[3][:], w2_d[0:P, :])
            nc.sync.dma_start(wstg[4][:], w2_d[P:C, :])
            nc.scalar.activation(w1s[0][:], wstg[0][:], AF.Copy)
            nc.scalar.activation(w1s[1][:], wstg[1][:], AF.Copy)
            nc.scalar.activation(w1s[2][:], wstg[2][:CY + 1, :], AF.Copy)
            nc.scalar.activation(w2s[0][:], wstg[3][:], AF.Copy)
            nc.scalar.activation(w2s[1][:], wstg[4][:], AF.Copy)
            nc.sync.dma_start(b1s[0][:], b1_d[0:P])
            nc.sync.dma_start(b1s[1][:], b1_d[P:C])
            nc.sync.dma_start(b2s[0][:], b2_d[0:P])
            nc.sync.dma_start(b2s[1][:], b2_d[P:C])

            with tc.tile_pool(name="s0sb", bufs=3) as sp, \
                 tc.tile_pool(name="s0ps", bufs=4, space="PSUM") as tps, \
                 tc.tile_pool(name="s0ps2", bufs=2, space="PSUM") as sps, \
                 tc.tile_pool(name="s0ps3", bufs=2, space="PSUM") as bps:
                # codebook: load, l2norm -> mn, transpose -> mnT, scale by RATE
                for kc in range(KC):
                    fwt = sp.tile([P, CD], f32, tag="fwt")
                    nc.sync.dma_start(fwt[:], fw_d[kc * P:(kc + 1) * P, :])
                    sq = sp.tile([P, C], f32, tag="sq")
                    ssq = sp.tile([P, 1], f32, tag="ssq")
                    nc.scalar.activation(sq[:], fwt[:, :C], AF.Square,
                                         accum_out=ssq[:])
                    nrm = sp.tile([P, 1], f32, tag="nrm")
                    nc.scalar.activation(nrm[:], ssq[:], AF.Sqrt)
                    rn = sp.tile([P, 1], f32, tag="rn")
                    nc.vector.reciprocal(rn[:], nrm[:])
                    mn = sp.tile([P, C], f32, tag="mn")
                    nc.vector.tensor_scalar_mul(mn[:], fwt[:, :C], rn[:])
                    for ci in range(NCC):
                        tp = tps.tile([P, P], f32, tag="tp")
                        nc.tensor.transpose(tp[:], mn[:, ci * P:(ci + 1) * P],
                                            ident[:])
                        if ci == 0:
                            nc.vector.tensor_copy(
                                mnT[ci][:, kc * P:(kc + 1) * P], tp[:])
                        else:
                            nc.scalar.activation(
                                mnT[ci][:, kc * P:(kc + 1) * P], tp[:],
                                AF.Copy)

                # x: load raw, build xyT (token-part, bf16), build xn (c-part)
                xraw = [mp.tile([P, HWN], f32, name=f"xraw{i}")
                        for i in range(NCC)]
                for ci in range(NCC):
                    nc.sync.dma_start(xraw[ci][:], xm[ci * P:(ci + 1) * P, :])
                for tt in range(NT):
                    tsl = slice(tt * P, (tt + 1) * P)
                    y_t = sp.tile([CY, P], f32, tag="y_t")
                    nc.sync.dma_start(y_t[:], ym[:, tsl])
                    tpb = tps.tile([P, CD], f32, tag="tp")
                    for ci in range(NCC):
                        nc.tensor.transpose(tpb[:, ci * P:(ci + 1) * P],
                                            xraw[ci][:, tsl], ident[:])
                    nc.tensor.transpose(tpb[:, C:CD], y_t[:],
                                        ident[:CY, :CY])
                    nc.scalar.activation(xyT[tt][:, :CD], tpb[:], AF.Copy)
                    nc.vector.memset(xyT[tt][:, CD:CDA], 1.0)

                # per-token 1/||x|| and xn = x * rinv
                for gs in range(NG2):
                    gsl = slice(gs * NGW, (gs + 1) * NGW)
                    ssp = sps.tile([1, NGW], f32, tag="ssp")
                    for ci in range(NCC):
                        xsq = sp.tile([P, NGW], f32r, tag="xsq")
                        nc.scalar.activation(xsq[:], xraw[ci][:, gsl],
                                             AF.Square)
                        nc.tensor.matmul(ssp[:], r(ones_col[:]), r(xsq[:]),
                                         start=(ci == 0), stop=(ci == NCC - 1))
                    srow = sp.tile([1, NGW], f32r, tag="srow")
                    nc.scalar.activation(srow[:], ssp[:], AF.Sqrt)
                    rbp = bps.tile([P, NGW], f32, tag="rbp")
                    nc.tensor.matmul(rbp[:], r(ones_row[:]), srow[:],
                                     start=True, stop=True)
                    rr_sb = sp.tile([P, NGW], f32, tag="rr_sb")
                    nc.vector.reciprocal(rr_sb[:], rbp[:])
                    for ci in range(NCC):
                        nc.vector.tensor_tensor(xn[ci][:, gsl],
                                                xraw[ci][:, gsl], rr_sb[:],
                                                OP.mult)

            # ---- stage 1: raw scores -> one-hot -> segment sums ----
            with tc.tile_pool(name="s1sc", bufs=4) as scp, \
                 tc.tile_pool(name="s1oh", bufs=GRP + 2) as ohp, \
                 tc.tile_pool(name="s1sm", bufs=3) as smp, \
                 tc.tile_pool(name="s1ps", bufs=3, space="PSUM") as sps1, \
                 tc.tile_pool(name="s1ps2", bufs=2, space="PSUM") as gps1:
                KH = K // 2  # 1024-wide score halves: 2-bank psum tiles
                for g in range(NT // GRP):
                    ohs = []
                    for t8 in range(GRP):
                        tt = g * GRP + t8
                        tsl = slice(tt * P, (tt + 1) * P)
                        scb = scp.tile([P, K], bf16, tag="scb")
                        for h in range(2):
                            scps = sps1.tile([P, KH], f32, tag="scps")
                            for ci in range(NCC):
                                for ns in range(KH // NGW):
                                    nsl = slice(ns * NGW, (ns + 1) * NGW)
                                    nc.tensor.matmul(
                                        scps[:, nsl],
                                        r(xn[ci][:, tsl]),
                                        r(mnT[ci][:, h * KH + ns * NGW:
                                                   h * KH + (ns + 1) * NGW]),
                                        start=(ci == 0), stop=(ci == NCC - 1))
                            nc.scalar.activation(scb[:, h * KH:(h + 1) * KH],
                                                 scps[:], AF.Copy)
                        rmx = smp.tile([P, 1], f32, tag="rmx")
                        nc.vector.tensor_reduce(rmx[:], scb[:], AX.X, OP.max)
                        oh = ohp.tile([P, K], bf16, tag="oh")
                        eq_eng = nc.gpsimd if (t8 % 2 == 0) else nc.vector
                        eq_eng.tensor_scalar(oh[:], scb[:], rmx[:], None,
                                             OP.is_equal)
                        ohs.append(oh)
                    for kc in range(KC):
                        ksl = slice(kc * P, (kc + 1) * P)
                        segp = gps1.tile([P, CDA], f32, tag="segp")
                        for t8 in range(GRP):
                            nc.tensor.matmul(segp[:], ohs[t8][:, ksl],
                                             xyT[g * GRP + t8][:],
                                             start=(t8 == 0),
                                             stop=(t8 == GRP - 1))
                        if g == 0:
                            nc.scalar.activation(sums[kc][:], segp[:], AF.Copy)
                        else:
                            nc.vector.tensor_tensor(sums[kc][:], sums[kc][:],
                                                    segp[:], OP.add)

            # ---- stage 2: all-reduce counts/sums, EMA update, l2norm ----
            mid.close()
            for kc in range(KC):
                nc.sync.dma_start(cc_in[kc * P:(kc + 1) * P, :], sums[kc][:])
            if single_core:
                # timeline-sim variant: model the collective as a local copy
                nc.sync.dma_start(cc_out[:, :], cc_in[:, :])
            else:
                nc.gpsimd.collective_compute(
                    "AllReduce", OP.add,
                    replica_groups=[list(range(N_CORES))],
                    ins=[cc_in.opt()], outs=[cc_out.opt()])
            PREG = 2
            s3ctx = ExitStack()
            ep = s3ctx.enter_context(tc.tile_pool(name="s3E", bufs=4))
            psE = s3ctx.enter_context(
                tc.tile_pool(name="psE", bufs=2, space="PSUM"))
            E_groups = {}

            def compute_E(g):
                gsl = slice(g * NGW, (g + 1) * NGW)
                Es = []
                for kc in range(KC):
                    scT = psE.tile([P, NGW], f32, tag="scT", name="scT")
                    for ci in range(NCC):
                        nc.tensor.matmul(
                            scT[:],
                            r(mnT[ci][:, kc * P:(kc + 1) * P]),
                            r(xn[ci][:, gsl]),
                            start=(ci == 0), stop=(ci == NCC - 1))
                    Et = ep.tile([P, NGW], bf16, tag=f"E{kc}", name="Et")
                    nc.scalar.activation(Et[:], scT[:], AF.Exp)
                    Es.append(Et)
                E_groups[g] = Es

            for g in range(PREG):
                compute_E(g)

            with tc.tile_pool(name="s2sb", bufs=3) as s2p:
                for kc in range(KC):
                    sr = s2p.tile([P, CDA], f32, tag="sr")
                    nc.sync.dma_start(sr[:], cc_out[kc * P:(kc + 1) * P, :])
                    cnt = s2p.tile([P, 1], f32, tag="cnt")
                    nc.vector.tensor_scalar_add(cnt[:], sr[:, CD:CDA],
                                                float(EPS_CNT))
                    rc = s2p.tile([P, 1], f32, tag="rc")
                    nc.vector.reciprocal(rc[:], cnt[:])
                    # nw_pre = fws (= feat_w*RATE) + (sums * rc) * (1-RATE)
                    em = s2p.tile([P, CD], f32, tag="em")
                    nc.vector.tensor_scalar_mul(em[:], sr[:, :CD], rc[:])
                    fwt2 = s2p.tile([P, CD], f32, tag="fwt2")
                    nc.sync.dma_start(fwt2[:], fw_d[kc * P:(kc + 1) * P, :])
                    fsc = s2p.tile([P, CD], f32, tag="fsc")
                    nc.vector.tensor_scalar_mul(fsc[:], fwt2[:], RATE)
                    npre = s2p.tile([P, CD], f32, tag="npre")
                    nc.vector.scalar_tensor_tensor(
                        npre[:], em[:], float(1.0 - RATE), fsc[:],
                        op0=OP.mult, op1=OP.add)
                    sq2 = s2p.tile([P, CD], f32, tag="sq2")
                    ssq2 = s2p.tile([P, 1], f32, tag="ssq2")
                    nc.gpsimd.tensor_tensor(sq2[:], npre[:], npre[:], OP.mult)
                    nc.vector.tensor_reduce(ssq2[:], sq2[:], AX.X, OP.add)
                    nr2 = s2p.tile([P, 1], f32, tag="nr2")
                    nc.scalar.activation(nr2[:], ssq2[:], AF.Sqrt)
                    rn2 = s2p.tile([P, 1], f32, tag="rn2")
                    nc.vector.reciprocal(rn2[:], nr2[:])
                    nc.vector.tensor_scalar_mul(nw[kc][:, :C],
                                                npre[:, :C], rn2[:])
                    nc.vector.tensor_scalar_mul(nw[kc][:, C + 1:CDA],
                                                npre[:, C:CD], rn2[:])
                    nc.scalar.activation(nw[kc][:, C:C + 1], ones_f32[:],
                                         AF.Copy)

            # ---- stage 3: softmax attention + MLP (transposed layout) ----
            with tc.tile_pool(name="s3sb", bufs=2) as s3p, \
                 tc.tile_pool(name="s3o", bufs=3) as s3o, \
                 tc.tile_pool(name="psA", bufs=3, space="PSUM") as psA, \
                 tc.tile_pool(name="psR", bufs=1, space="PSUM") as psR, \
                 tc.tile_pool(name="psM", bufs=2, space="PSUM") as psM:
                mchunks = [(0, P), (P, P), (2 * P, CDA - 2 * P)]
                for g in range(NG2):
                    gsl = slice(g * NGW, (g + 1) * NGW)
                    if g not in E_groups:
                        compute_E(g)
                    Es = E_groups.pop(g)
                    atts = []
                    for mi, (m0, mw) in enumerate(mchunks):
                        att = psA.tile([P, NGW], f32, tag="att")
                        for kc in range(KC):
                            nc.tensor.matmul(att[:mw, :],
                                             nw[kc][:, m0:m0 + mw],
                                             Es[kc][:],
                                             start=(kc == 0),
                                             stop=(kc == KC - 1))
                        atts.append(att)
                    # nw col 256 is the ones column, so atts[2] row 0 is
                    # sumexp (partition-0-aligned for PSUM reads).
                    se_sb = s3p.tile([1, NGW], f32r, tag="se_sb")
                    nc.scalar.activation(se_sb[:], atts[2][0:1, :], AF.Copy)
                    rb = psR.tile([P, NGW], f32, tag="rb")
                    nc.tensor.matmul(rb[:], r(ones_row[:]), se_sb[:],
                                     start=True, stop=True)
                    rb_sb = s3p.tile([P, NGW], f32, tag="rb_sb")
                    nc.vector.reciprocal(rb_sb[:], rb[:])
                    o2 = [s3p.tile([P, NGW], f32r, tag=f"o2_{i}",
                                   name=f"o2_{i}")
                          for i in range(2)]
                    o2y5 = s3p.tile([CY + 1, NGW], f32r, tag="o2y5")
                    for mi in range(2):
                        nc.vector.tensor_tensor(o2[mi][:], atts[mi][:],
                                                rb_sb[:], OP.mult)
                    nc.vector.tensor_tensor(o2y5[:], atts[2][:CY + 1, :],
                                            rb_sb[:CY + 1, :], OP.mult)
                    o2all = o2 + [o2y5]
                    # MLP: hT = gelu(w1.T @ out2T + b1); oT = w2.T @ hT + b2
                    hT = []
                    ksegs = [(0, P), (P, P), (2 * P, CY + 1)]
                    for hm in range(2):
                        hps = psM.tile([P, NGW], f32, tag="mlp")
                        for j, (k0, kw) in enumerate(ksegs):
                            nc.tensor.matmul(
                                hps[:],
                                r(w1s[j][:, hm * P:(hm + 1) * P]),
                                r(o2all[j][:kw, :]),
                                start=(j == 0), stop=(j == 2))
                        # |h| < ~1e-2 here, so tanh-gelu == x*(0.5 +
                        # 0.3989423*x) to ~1e-10 abs; avoids ACT table loads
                        hx = s3p.tile([P, NGW], f32, tag=f"hx{hm}")
                        nc.scalar.activation(hx[:], hps[:], AF.Identity,
                                             bias=b1s[hm][:])
                        t1 = s3p.tile([P, NGW], f32, tag="t1")
                        nc.vector.tensor_scalar(t1[:], hx[:],
                                                0.3989422804014327, 0.5,
                                                OP.mult, OP.add)
                        ht = s3p.tile([P, NGW], f32r, tag=f"hT{hm}")
                        nc.vector.tensor_tensor(ht[:], t1[:], hx[:], OP.mult)
                        hT.append(ht)
                    for mo in range(2):
                        ops_ = psM.tile([P, NGW], f32, tag="mlp")
                        for kc2 in range(2):
                            nc.tensor.matmul(
                                ops_[:],
                                r(w2s[kc2][:, mo * P:(mo + 1) * P]),
                                r(hT[kc2][:]),
                                start=(kc2 == 0), stop=(kc2 == 1))
                        outt = s3o.tile([P, NGW], f32, tag="outt")
                        nc.vector.tensor_scalar_add(outt[:], ops_[:],
                                                    b2s[mo][:])
                        nc.sync.dma_start(om[mo * P:(mo + 1) * P, gsl],
                                          outt[:])
            s3ctx.close()

    nc.compile()
    return nc


def _get_nc():
    if "nc" not in _CACHE:
        _CACHE["nc"] = _build_nc()
    return _CACHE["nc"]


def kernel(x, y, feat_w, w1, b1, w2, b2):
    from concourse.bass_utils import run_bass_kernel_spmd

    nc = _get_nc()
    in_maps = []
    for m in range(N_CORES):
        in_maps.append({
            "xm": np.ascontiguousarray(x[m].reshape(C, HWN), dtype=np.float32),
            "ym": np.ascontiguousarray(y[m].reshape(CY, HWN),
                                       dtype=np.float32),
            "feat_w": np.ascontiguousarray(feat_w, dtype=np.float32),
            "w1": np.ascontiguousarray(w1, dtype=np.float32),
            "b1": np.ascontiguousarray(b1, dtype=np.float32),
            "w2": np.ascontiguousarray(w2, dtype=np.float32),
            "b2": np.ascontiguousarray(b2, dtype=np.float32),
        })
    res = run_bass_kernel_spmd(nc, in_maps, core_ids=list(range(N_CORES)))
    out = np.stack([res.results[m]["om"].reshape(C, H, W)
                    for m in range(N_CORES)])
    return out.astype(np.float32)



# revision 8
# speedup vs baseline: 1.0863x; 1.0863x over previous
"""Trainium2 Bass kernel for nn_MemoryN2N (vq_codebook).

Self-contained: hardcodes shapes/sharding. Data-parallel over the
n = b*h*w token axis: core m processes batch element m (4096 tokens).
Codebook + MLP weights replicated; segment-sum counts/sums all-reduced.

Segment sums use fp8e4 DoubleRow matmuls (256-deep contraction at 0.5
cycles/row); the one-hot is exact in fp8 and the xy quantization only
perturbs the 0.001-weighted EMA blend.
"""

import numpy as np

# -- problem constants (hardcoded from the problem spec) --
B, C, H, W, K = 8, 256, 64, 64, 2048
CY = 4                 # y channels
CD = C + CY            # 260
CDA = CD + 1           # 261 (+ ones column for counts / sumexp)
HWN = H * W            # 4096 tokens per core
P = 128
KC = K // P            # 16 codebook chunks
NCC = C // P           # 2 channel chunks
NT = HWN // P          # 32 token tiles (pass 1)
NPAIR = NT // 2        # 16 token-tile pairs (fp8 DoubleRow segment)
RGRP = 8               # pairs per segment round
NGW = 512              # pass-2 token group width
NG2 = HWN // NGW       # 8 pass-2 groups
N_CORES = 8
RATE = 0.999
EPS_CNT = 1e-6
DH0, DH1 = 134, CDA - 134   # rhs free-dim halves for DoubleRow (2*dw <= 512)

_CACHE = {}


def _build_nc(single_core=False):
    import concourse.bacc as bacc
    import concourse.mybir as mybir
    import concourse.tile as tile

    f32 = mybir.dt.float32
    f32r = mybir.dt.float32r
    bf16 = mybir.dt.bfloat16
    fp8 = mybir.dt.float8e4
    i32 = mybir.dt.int32
    AF = mybir.ActivationFunctionType
    OP = mybir.AluOpType
    AX = mybir.AxisListType
    PM = mybir.MatmulPerfMode

    nc = bacc.Bacc("TRN2", target_bir_lowering=False, debug=False,
                   num_devices=1 if single_core else N_CORES)

    xm = nc.dram_tensor("xm", [C, HWN], f32, kind="ExternalInput").ap()
    ym = nc.dram_tensor("ym", [CY, HWN], f32, kind="ExternalInput").ap()
    fw_d = nc.dram_tensor("feat_w", [K, CD], f32, kind="ExternalInput").ap()
    w1_d = nc.dram_tensor("w1", [CD, C], f32, kind="ExternalInput").ap()
    b1_d = nc.dram_tensor("b1", [C], f32, kind="ExternalInput").ap()
    w2_d = nc.dram_tensor("w2", [C, C], f32, kind="ExternalInput").ap()
    b2_d = nc.dram_tensor("b2", [C], f32, kind="ExternalInput").ap()
    om = nc.dram_tensor("om", [C, HWN], f32, kind="ExternalOutput").ap()

    def r(ap):  # relaxed-fp32 view for PE matmuls
        if ap.dtype == f32r:
            return ap
        return ap.bitcast(f32r)

    from contextlib import ExitStack

    with tile.TileContext(nc) as tc:
        with tc.tile_pool(name="persist", bufs=1) as pp, \
             tc.tile_pool(name="dram", bufs=1, space="DRAM") as dp:
            # ---- persistent tiles ----
            xn = [pp.tile([P, HWN], f32r, name=f"xn{i}") for i in range(NCC)]
            mnT = [pp.tile([P, K], f32r, name=f"mnT{i}") for i in range(NCC)]
            nw = [pp.tile([P, CDA], bf16, name=f"nw{i}") for i in range(KC)]
            sums = [pp.tile([P, CDA], f32, name=f"sums{i}")
                    for i in range(KC)]
            # xy8 pair tiles released after stage 1
            mid = ExitStack()
            mp = mid.enter_context(tc.tile_pool(name="midp", bufs=1))
            xy8 = [mp.tile([P, 2, CDA], fp8, name=f"xy8_{i}")
                   for i in range(NPAIR)]
            w1s = [pp.tile([P, C], f32r, name="w1s0"),
                   pp.tile([P, C], f32r, name="w1s1"),
                   pp.tile([CY + 1, C], f32r, name="w1s2")]
            w2s = [pp.tile([P, C], f32r, name=f"w2s{i}") for i in range(2)]
            b1s = [pp.tile([P, 1], f32, name=f"b1s{i}") for i in range(2)]
            b2s = [pp.tile([P, 1], f32, name=f"b2s{i}") for i in range(2)]
            ones_col = pp.tile([P, 1], f32r, name="ones_col")
            ones_row = pp.tile([1, P], f32r, name="ones_row")
            ident = pp.tile([P, P], f32, name="ident")

            cc_in = dp.tile([K, CDA], f32, name="cc_in")
            cc_out = dp.tile([K, CDA], f32, name="cc_out",
                             addr_space="Shared")

            # ---- stage 0: constants, weights, codebook prep ----
            ones_f32 = pp.tile([P, 1], f32, name="ones_f32")
            orow_f32 = pp.tile([1, P], f32, name="orow_f32")
            nc.vector.memset(ones_f32[:], 1.0)
            nc.vector.memset(orow_f32[:], 1.0)
            nc.scalar.activation(ones_col[:], ones_f32[:], AF.Copy)
            nc.scalar.activation(ones_row[:], orow_f32[:], AF.Copy)
            iid = pp.tile([P, P], i32, name="iid")
            nc.gpsimd.iota(iid[:], pattern=[[1, P]], base=0,
                           channel_multiplier=-1)
            nc.gpsimd.tensor_scalar(ident[:], iid[:], 0, None, OP.is_equal)

            wstg = [pp.tile([P, C], f32, name=f"wstg{i}") for i in range(5)]
            nc.sync.dma_start(wstg[0][:], w1_d[0:P, :])
            nc.sync.dma_start(wstg[1][:], w1_d[P:2 * P, :])
            nc.vector.memset(wstg[2][0:1, :], 0.0)
            nc.sync.dma_start(wstg[2][1:CY + 1, :], w1_d[2 * P:CD, :])
            nc.sync.dma_start(wstg[3][:], w2_d[0:P, :])
            nc.sync.dma_start(wstg[4][:], w2_d[P:C, :])
            nc.scalar.activation(w1s[0][:], wstg[0][:], AF.Copy)
            nc.scalar.activation(w1s[1][:], wstg[1][:], AF.Copy)
            nc.scalar.activation(w1s[2][:], wstg[2][:CY + 1, :], AF.Copy)
            nc.scalar.activation(w2s[0][:], wstg[3][:], AF.Copy)
            nc.scalar.activation(w2s[1][:], wstg[4][:], AF.Copy)
            nc.sync.dma_start(b1s[0][:], b1_d[0:P])
            nc.sync.dma_start(b1s[1][:], b1_d[P:C])
            nc.sync.dma_start(b2s[0][:], b2_d[0:P])
            nc.sync.dma_start(b2s[1][:], b2_d[P:C])

            s0ctx = ExitStack()
            xrp = s0ctx.enter_context(tc.tile_pool(name="s0xr", bufs=1))
            with tc.tile_pool(name="s0sb", bufs=3) as sp, \
                 tc.tile_pool(name="s0ps", bufs=4, space="PSUM") as tps, \
                 tc.tile_pool(name="s0ps2", bufs=2, space="PSUM") as sps, \
                 tc.tile_pool(name="s0ps3", bufs=2, space="PSUM") as bps:
                # codebook: load, l2norm -> mn, transpose -> mnT
                for kc in range(KC):
                    fwt = sp.tile([P, CD], f32, tag="fwt")
                    nc.sync.dma_start(fwt[:], fw_d[kc * P:(kc + 1) * P, :])
                    sq = sp.tile([P, C], f32, tag="sq")
                    ssq = sp.tile([P, 1], f32, tag="ssq")
                    nc.scalar.activation(sq[:], fwt[:, :C], AF.Square,
                                         accum_out=ssq[:])
                    nrm = sp.tile([P, 1], f32, tag="nrm")
                    nc.scalar.activation(nrm[:], ssq[:], AF.Sqrt)
                    rn = sp.tile([P, 1], f32, tag="rn")
                    nc.vector.reciprocal(rn[:], nrm[:])
                    mn = sp.tile([P, C], f32, tag="mn")
                    nc.vector.tensor_scalar_mul(mn[:], fwt[:, :C], rn[:])
                    for ci in range(NCC):
                        tp = tps.tile([P, P], f32, tag="tp")
                        nc.tensor.transpose(tp[:], mn[:, ci * P:(ci + 1) * P],
                                            ident[:])
                        if ci == 0:
                            nc.vector.tensor_copy(
                                mnT[ci][:, kc * P:(kc + 1) * P], tp[:])
                        else:
                            nc.scalar.activation(
                                mnT[ci][:, kc * P:(kc + 1) * P], tp[:],
                                AF.Copy)

                # x: load raw, build xy8 (paired fp8), build xn (c-part)
                xraw = [xrp.tile([P, HWN], f32, name=f"xraw{i}")
                        for i in range(NCC)]
                for ci in range(NCC):
                    nc.sync.dma_start(xraw[ci][:], xm[ci * P:(ci + 1) * P, :])
                for tt in range(NT):
                    tsl = slice(tt * P, (tt + 1) * P)
                    pr, j = tt // 2, tt % 2
                    y_t = sp.tile([CY, P], f32, tag="y_t")
                    nc.sync.dma_start(y_t[:], ym[:, tsl])
                    tpb = tps.tile([P, CD], f32, tag="tp")
                    for ci in range(NCC):
                        nc.tensor.transpose(tpb[:, ci * P:(ci + 1) * P],
                                            xraw[ci][:, tsl], ident[:])
                    nc.tensor.transpose(tpb[:, C:CD], y_t[:],
                                        ident[:CY, :CY])
                    nc.scalar.activation(xy8[pr][:, j, :CD], tpb[:], AF.Copy)
                    nc.vector.memset(xy8[pr][:, j, CD:CDA], 1.0)

                # per-token 1/||x|| and xn = x * rinv
                for gs in range(NG2):
                    gsl = slice(gs * NGW, (gs + 1) * NGW)
                    ssp = sps.tile([1, NGW], f32, tag="ssp")
                    for ci in range(NCC):
                        xsq = sp.tile([P, NGW], f32r, tag="xsq")
                        nc.scalar.activation(xsq[:], xraw[ci][:, gsl],
                                             AF.Square)
                        nc.tensor.matmul(ssp[:], r(ones_col[:]), r(xsq[:]),
                                         start=(ci == 0), stop=(ci == NCC - 1))
                    srow = sp.tile([1, NGW], f32r, tag="srow")
                    nc.scalar.activation(srow[:], ssp[:], AF.Sqrt)
                    rbp = bps.tile([P, NGW], f32, tag="rbp")
                    nc.tensor.matmul(rbp[:], r(ones_row[:]), srow[:],
                                     start=True, stop=True)
                    rr_sb = sp.tile([P, NGW], f32, tag="rr_sb")
                    nc.vector.reciprocal(rr_sb[:], rbp[:])
                    for ci in range(NCC):
                        nc.vector.tensor_tensor(xn[ci][:, gsl],
                                                xraw[ci][:, gsl], rr_sb[:],
                                                OP.mult)
            s0ctx.close()   # free xraw

            # ---- stage 1: raw scores -> one-hot -> segment sums (fp8 DR) --
            sums2 = None
            with tc.tile_pool(name="s1sc", bufs=4) as scp, \
                 tc.tile_pool(name="s1oh", bufs=RGRP + 2) as ohp, \
                 tc.tile_pool(name="s1sm", bufs=4) as smp, \
                 tc.tile_pool(name="s1s2", bufs=1) as sm2p, \
                 tc.tile_pool(name="s1ps", bufs=2, space="PSUM") as sps1, \
                 tc.tile_pool(name="s1ps2", bufs=4, space="PSUM") as gps1:
                KH = K // 2  # 1024-wide score halves: 2-bank psum tiles
                for rnd in range(NT // (2 * RGRP)):
                    oh8s = []
                    for p8 in range(RGRP):
                        pr = rnd * RGRP + p8
                        oh8 = ohp.tile([P, 2, K], fp8, tag="oh8")
                        for j in range(2):
                            tt = pr * 2 + j
                            tsl = slice(tt * P, (tt + 1) * P)
                            scb = scp.tile([P, K], bf16, tag="scb")
                            for h in range(2):
                                scps = sps1.tile([P, KH], f32, tag="scps")
                                for ci in range(NCC):
                                    for ns in range(KH // NGW):
                                        nsl = slice(ns * NGW, (ns + 1) * NGW)
                                        nc.tensor.matmul(
                                            scps[:, nsl],
                                            r(xn[ci][:, tsl]),
                                            r(mnT[ci][:, h * KH + ns * NGW:
                                                       h * KH + (ns + 1) * NGW]),
                                            start=(ci == 0),
                                            stop=(ci == NCC - 1))
                                nc.scalar.activation(
                                    scb[:, h * KH:(h + 1) * KH],
                                    scps[:], AF.Copy)
                            # row max over K via bf16 max-tree + reduce
                            mx1 = smp.tile([P, KH], bf16, tag="mx1")
                            nc.vector.tensor_tensor(mx1[:], scb[:, :KH],
                                                    scb[:, KH:], OP.max)
                            mx2 = smp.tile([P, KH // 2], bf16, tag="mx2")
                            nc.vector.tensor_tensor(mx2[:], mx1[:, :KH // 2],
                                                    mx1[:, KH // 2:], OP.max)
                            rmx = smp.tile([P, 1], f32, tag="rmx")
                            nc.vector.tensor_reduce(rmx[:], mx2[:], AX.X,
                                                    OP.max)
                            eq_eng = nc.gpsimd if (j == 0) else nc.vector
                            eq_eng.tensor_scalar(oh8[:, j, :], scb[:],
                                                 rmx[:], None, OP.is_equal)
                        oh8s.append(oh8)
                    # fp8 DoubleRow segment sums: 32 k-chunks of 64
                    if rnd == 0:
                        dst = sums
                    else:
                        if sums2 is None:
                            sums2 = [sm2p.tile([P, CDA], f32,
                                               name=f"sm2_{i}")
                                     for i in range(KC)]
                        dst = sums2
                    for k64 in range(K // 64):
                        segp = gps1.tile([64, CDA], f32, tag="segp")
                        for p8 in range(RGRP):
                            pr = rnd * RGRP + p8
                            for d0, dw in ((0, DH0), (DH0, DH1)):
                                nc.tensor.matmul(
                                    segp[:, d0:d0 + dw],
                                    oh8s[p8][:, :, k64 * 64:(k64 + 1) * 64],
                                    xy8[pr][:, :, d0:d0 + dw],
                                    start=(p8 == 0), stop=(p8 == RGRP - 1),
                                    perf_mode=PM.DoubleRow,
                                    tile_position=(0, 0))
                        kc, half = k64 // 2, k64 % 2
                        nc.scalar.activation(
                            dst[kc][half * 64:(half + 1) * 64, :],
                            segp[:], AF.Copy)
                # fold round-1 partials into sums
                for kc in range(KC):
                    nc.vector.tensor_tensor(sums[kc][:], sums[kc][:],
                                            sums2[kc][:], OP.add)

            # ---- stage 2: all-reduce counts/sums, EMA update, l2norm ----
            mid.close()
            for kc in range(KC):
                nc.sync.dma_start(cc_in[kc * P:(kc + 1) * P, :], sums[kc][:])
            if single_core:
                # timeline-sim variant: model the collective as a local copy
                nc.sync.dma_start(cc_out[:, :], cc_in[:, :])
            else:
                nc.gpsimd.collective_compute(
                    "AllReduce", OP.add,
                    replica_groups=[list(range(N_CORES))],
                    ins=[cc_in.opt()], outs=[cc_out.opt()])
            PREG = 2
            s3ctx = ExitStack()
            ep = s3ctx.enter_context(tc.tile_pool(name="s3E", bufs=4))
            psE = s3ctx.enter_context(
                tc.tile_pool(name="psE", bufs=2, space="PSUM"))
            E_groups = {}

            def compute_E(g):
                gsl = slice(g * NGW, (g + 1) * NGW)
                Es = []
                for kc in range(KC):
                    scT = psE.tile([P, NGW], f32, tag="scT", name="scT")
                    for ci in range(NCC):
                        nc.tensor.matmul(
                            scT[:],
                            r(mnT[ci][:, kc * P:(kc + 1) * P]),
                            r(xn[ci][:, gsl]),
                            start=(ci == 0), stop=(ci == NCC - 1))
                    Et = ep.tile([P, NGW], bf16, tag=f"E{kc}", name="Et")
                    nc.scalar.activation(Et[:], scT[:], AF.Exp)
                    Es.append(Et)
                E_groups[g] = Es

            for g in range(PREG):
                compute_E(g)

            with tc.tile_pool(name="s2sb", bufs=3) as s2p, \
                 tc.tile_pool(name="s2np", bufs=1) as s2np:
                # batched norm: one sqrt + one reciprocal for all 16 chunks
                npres = []
                ssqB = s2np.tile([P, KC], f32, name="ssqB")
                for kc in range(KC):
                    sr = s2p.tile([P, CDA], f32, tag="sr")
                    nc.sync.dma_start(sr[:], cc_out[kc * P:(kc + 1) * P, :])
                    cnt = s2p.tile([P, 1], f32, tag="cnt")
                    nc.vector.tensor_scalar_add(cnt[:], sr[:, CD:CDA],
                                                float(EPS_CNT))
                    rc = s2p.tile([P, 1], f32, tag="rc")
                    nc.vector.reciprocal(rc[:], cnt[:])
                    # nw_pre = fws (= feat_w*RATE) + (sums * rc) * (1-RATE)
                    em = s2p.tile([P, CD], f32, tag="em")
                    nc.vector.tensor_scalar_mul(em[:], sr[:, :CD], rc[:])
                    fwt2 = s2p.tile([P, CD], f32, tag="fwt2")
                    nc.sync.dma_start(fwt2[:], fw_d[kc * P:(kc + 1) * P, :])
                    fsc = s2p.tile([P, CD], f32, tag="fsc")
                    nc.vector.tensor_scalar_mul(fsc[:], fwt2[:], RATE)
                    npre = s2np.tile([P, CD], f32, name=f"npre{kc}")
                    nc.vector.scalar_tensor_tensor(
                        npre[:], em[:], float(1.0 - RATE), fsc[:],
                        op0=OP.mult, op1=OP.add)
                    sq2 = s2p.tile([P, CD], f32, tag="sq2")
                    nc.gpsimd.tensor_tensor(sq2[:], npre[:], npre[:], OP.mult)
                    nc.vector.tensor_reduce(ssqB[:, kc:kc + 1], sq2[:], AX.X,
                                            OP.add)
                    npres.append(npre)
                nrB = s2np.tile([P, KC], f32, name="nrB")
                nc.scalar.activation(nrB[:], ssqB[:], AF.Sqrt)
                rnB = s2np.tile([P, KC], f32, name="rnB")
                nc.vector.reciprocal(rnB[:], nrB[:])
                for kc in range(KC):
                    nc.vector.tensor_scalar_mul(nw[kc][:, :C],
                                                npres[kc][:, :C],
                                                rnB[:, kc:kc + 1])
                    nc.vector.tensor_scalar_mul(nw[kc][:, C + 1:CDA],
                                                npres[kc][:, C:CD],
                                                rnB[:, kc:kc + 1])
                    nc.scalar.activation(nw[kc][:, C:C + 1], ones_f32[:],
                                         AF.Copy)

            # ---- stage 3: softmax attention + MLP (transposed layout) ----
            with tc.tile_pool(name="s3sb", bufs=2) as s3p, \
                 tc.tile_pool(name="s3o", bufs=3) as s3o, \
                 tc.tile_pool(name="psA", bufs=3, space="PSUM") as psA, \
                 tc.tile_pool(name="psR", bufs=1, space="PSUM") as psR, \
                 tc.tile_pool(name="psM", bufs=2, space="PSUM") as psM:
                mchunks = [(0, P), (P, P), (2 * P, CDA - 2 * P)]
                for g in range(NG2):
                    gsl = slice(g * NGW, (g + 1) * NGW)
                    if g not in E_groups:
                        compute_E(g)
                    Es = E_groups.pop(g)
                    atts = []
                    for mi, (m0, mw) in enumerate(mchunks):
                        att = psA.tile([P, NGW], f32, tag="att")
                        for kc in range(KC):
                            nc.tensor.matmul(att[:mw, :],
                                             nw[kc][:, m0:m0 + mw],
                                             Es[kc][:],
                                             start=(kc == 0),
                                             stop=(kc == KC - 1))
                        atts.append(att)
                    # nw col 256 is the ones column, so atts[2] row 0 is
                    # sumexp (partition-0-aligned for PSUM reads).
                    se_sb = s3p.tile([1, NGW], f32r, tag="se_sb")
                    nc.scalar.activation(se_sb[:], atts[2][0:1, :], AF.Copy)
                    rb = psR.tile([P, NGW], f32, tag="rb")
                    nc.tensor.matmul(rb[:], r(ones_row[:]), se_sb[:],
                                     start=True, stop=True)
                    rb_sb = s3p.tile([P, NGW], f32, tag="rb_sb")
                    nc.vector.reciprocal(rb_sb[:], rb[:])
                    o2 = [s3p.tile([P, NGW], f32r, tag=f"o2_{i}",
                                   name=f"o2_{i}")
                          for i in range(2)]
                    o2y5 = s3p.tile([CY + 1, NGW], f32r, tag="o2y5")
                    for mi in range(2):
                        nc.vector.tensor_tensor(o2[mi][:], atts[mi][:],
                                                rb_sb[:], OP.mult)
                    nc.vector.tensor_tensor(o2y5[:], atts[2][:CY + 1, :],
                                            rb_sb[:CY + 1, :], OP.mult)
                    o2all = o2 + [o2y5]
                    # MLP: hT = gelu(w1.T @ out2T + b1); oT = w2.T @ hT + b2
                    hT = []
                    ksegs = [(0, P), (P, P), (2 * P, CY + 1)]
                    for hm in range(2):
                        hps = psM.tile([P, NGW], f32, tag="mlp")
                        for jk, (k0, kw) in enumerate(ksegs):
                            nc.tensor.matmul(
                                hps[:],
                                r(w1s[jk][:, hm * P:(hm + 1) * P]),
                                r(o2all[jk][:kw, :]),
                                start=(jk == 0), stop=(jk == 2))
                        # |h| < ~1e-2 here, so tanh-gelu == x*(0.5 +
                        # 0.3989423*x) to ~1e-10 abs; avoids ACT table loads
                        hx = s3p.tile([P, NGW], f32, tag=f"hx{hm}")
                        nc.scalar.activation(hx[:], hps[:], AF.Identity,
                                             bias=b1s[hm][:])
                        t1 = s3p.tile([P, NGW], f32, tag="t1")
                        nc.vector.tensor_scalar(t1[:], hx[:],
                                                0.3989422804014327, 0.5,
                                                OP.mult, OP.add)
                        ht = s3p.tile([P, NGW], f32r, tag=f"hT{hm}")
                        nc.vector.tensor_tensor(ht[:], t1[:], hx[:], OP.mult)
                        hT.append(ht)
                    for mo in range(2):
                        ops_ = psM.tile([P, NGW], f32, tag="mlp")
                        for kc2 in range(2):
                            nc.tensor.matmul(
                                ops_[:],
                                r(w2s[kc2][:, mo * P:(mo + 1) * P]),
                                r(hT[kc2][:]),
                                start=(kc2 == 0), stop=(kc2 == 1))
                        outt = s3o.tile([P, NGW], f32, tag="outt")
                        nc.vector.tensor_scalar_add(outt[:], ops_[:],
                                                    b2s[mo][:])
                        nc.sync.dma_start(om[mo * P:(mo + 1) * P, gsl],
                                          outt[:])
            s3ctx.close()

    nc.compile()
    return nc


def _get_nc():
    if "nc" not in _CACHE:
        _CACHE["nc"] = _build_nc()
    return _CACHE["nc"]


def kernel(x, y, feat_w, w1, b1, w2, b2):
    from concourse.bass_utils import run_bass_kernel_spmd

    nc = _get_nc()
    in_maps = []
    for m in range(N_CORES):
        in_maps.append({
            "xm": np.ascontiguousarray(x[m].reshape(C, HWN), dtype=np.float32),
            "ym": np.ascontiguousarray(y[m].reshape(CY, HWN),
                                       dtype=np.float32),
            "feat_w": np.ascontiguousarray(feat_w, dtype=np.float32),
            "w1": np.ascontiguousarray(w1, dtype=np.float32),
            "b1": np.ascontiguousarray(b1, dtype=np.float32),
            "w2": np.ascontiguousarray(w2, dtype=np.float32),
            "b2": np.ascontiguousarray(b2, dtype=np.float32),
        })
    res = run_bass_kernel_spmd(nc, in_maps, core_ids=list(range(N_CORES)))
    out = np.stack([res.results[m]["om"].reshape(C, H, W)
                    for m in range(N_CORES)])
    return out.astype(np.float32)
